# revision 23
# baseline (speedup 1.0000x reference)
"""GAT-style attention-diagonal kernel for Trainium2 (Bass/Tile), 8-core SPMD.

Reference computation (per (b,t) slice, x:[N,F]):
    Q = x@Wq + bq; K = x@Wk + bk; V = x@Wv + bv
    s = Q @ K.T / sqrt(F)            # [N,N]
    a = softmax(s, axis=-1)
    out = diag(a)[:, None] * V       # only the softmax diagonal is needed

Sharding: data-parallel on the fused B*T axis (48 slices -> 6 per core),
weights replicated.

v2 dataflow (fused path, bq=bk=0), per core per slice:
  - x loaded fp32, cast to bf16 on DVE, transposed to XT [f,n] by the DMA
    x-bar engine (dma_start_transpose; layout matches the [fi,fo] chunk
    convention exactly) -- no PE transposes, no PSUM round trip
  - one-time M = Wq @ Wk.T on device (bf16); XMT[g,n] = M.T @ XT per slice
    (K projection eliminated);  V[n,g] JIT per row chunk
  - scores = XMT.T @ XT: bulk in fp8(e4m3) with DoubleRow perf mode
    (2 k-subtiles per pass -> ~1.8x PE throughput on the N^2 block); the
    128-wide diagonal stripe is then OVERWRITTEN by a bf16 matmul group so
    the softmax numerator exp(s_nn) keeps ~0.3% accuracy while the
    denominator (a 1024-term sum) averages the fp8 noise away
  - exp fused with row-sum on ScalarE over the whole [128,1024] PSUM pair;
    diagonal via identity-mask tensor_tensor_reduce on DVE
  - out row chunk = (exp(s_nn)/rowsum) * V, scaled straight from PSUM

All matmul accumulation is fp32 in PSUM. Expected end-to-end rel err ~0.5%
(tolerance 2e-2).
"""

import numpy as np

B, T, N, F = 4, 12, 1024, 512
NCORES = 8
S = (B * T) // NCORES  # 6 slices per core
P = 128
NO = N // P   # 8 row chunks per slice
FO = F // P   # 4 f chunks
GO = F // P   # 4 g chunks
MH = N // 512  # 2 halves of the scores free axis
SCALE = float(1.0 / np.sqrt(np.float32(F)))

_CACHE: dict = {}


def build_program(n_slices: int = S, repeats: int = 1, fused_qk: bool = True):
    if not fused_qk:
        return build_program_v1(n_slices, repeats, fused_qk)
    return build_program_v2(n_slices, repeats)


def build_program_v2(
    n_slices: int = S,
    repeats: int = 1,
    use_dr: bool = True,
    debug: str = "",
):
    """debug flags (comma-separated, timing-only variants):
    noexp    - skip exp/diag/stats; scale V by 1.0
    noscores - skip the scores matmuls entirely (implies noexp)
    nocasts  - skip the fp8 cast instructions (DR reads uninitialized tiles)
    dveqt    - do the qt PSUM->SBUF bf16 copies on DVE instead of ScalarE
    sp3      - scores PSUM pool bufs=3 and shared V/proj pool (old layout)
    goinner  - projection loop order for go: for nh: (old layout)
    """
    import concourse.bass as bass
    import concourse.tile as tile
    from concourse import bacc, mybir
    from concourse.masks import make_identity
    from contextlib import ExitStack

    f32 = mybir.dt.float32
    bf16 = mybir.dt.bfloat16
    f8 = mybir.dt.float8e4
    EXP = mybir.ActivationFunctionType.Exp
    COPYF = mybir.ActivationFunctionType.Identity
    AX = mybir.AxisListType.X
    OP = mybir.AluOpType
    DR = mybir.MatmulPerfMode.DoubleRow

    nc = bacc.Bacc(trn_type="TRN2", target_bir_lowering=False, debug=False)
    # inputs arrive pre-cast to bf16 (host-side quantize; the kernel's own
    # matmuls are bf16/fp8 anyway) so the x-bar DMA transpose can read DRAM
    # directly (2-byte dtype requirement) and input DMA traffic halves
    x_d = nc.dram_tensor("x", [n_slices, N, F], bf16, kind="ExternalInput").ap()
    wq_d = nc.dram_tensor("wq", [F, F], bf16, kind="ExternalInput").ap()
    wk_d = nc.dram_tensor("wk", [F, F], bf16, kind="ExternalInput").ap()
    wv_d = nc.dram_tensor("wv", [F, F], bf16, kind="ExternalInput").ap()
    out_d = nc.dram_tensor("out", [n_slices, N, F], f32, kind="ExternalOutput").ap()

    with tile.TileContext(nc) as tc, ExitStack() as ctx:
        consts = ctx.enter_context(tc.tile_pool(name="consts", bufs=1))
        xt_pool = ctx.enter_context(tc.tile_pool(name="xt", bufs=2))
        x8_pool = ctx.enter_context(tc.tile_pool(name="x8", bufs=2))
        qt_pool = ctx.enter_context(tc.tile_pool(name="qt", bufs=2))
        q8_pool = ctx.enter_context(tc.tile_pool(name="q8", bufs=2))
        outp = ctx.enter_context(tc.tile_pool(name="outp", bufs=3))
        dscr = ctx.enter_context(tc.tile_pool(name="dscr", bufs=2))
        stats = ctx.enter_context(tc.tile_pool(name="stats", bufs=6))
        dbg0 = set(debug.split(",")) if debug else set()
        if "sp3" in dbg0:
            # 8 banks = pp(2x1, shared proj+V) + sp(3x2)
            pp = ctx.enter_context(tc.tile_pool(name="pp", bufs=2, space="PSUM"))
            vp = pp
            sp = ctx.enter_context(tc.tile_pool(name="sp", bufs=3, space="PSUM"))
        else:
            # 8 banks = pp(2x1 proj) + vp(2x1 V) + sp(2x2 scores)
            pp = ctx.enter_context(tc.tile_pool(name="pp", bufs=2, space="PSUM"))
            vp = ctx.enter_context(tc.tile_pool(name="vp", bufs=2, space="PSUM"))
            sp = ctx.enter_context(tc.tile_pool(name="sp", bufs=2, space="PSUM"))

        ident = consts.tile([P, P], f32, name="ident", tag="ident")
        make_identity(nc, ident[:])

        # ---- weights: Wv loaded [fi,fo,g]; WqT/WkT via one DRAM x-bar
        # transpose each; then M = Wq @ Wk.T on PE ----
        wv_sb = consts.tile([P, FO, F], bf16, name="wv_sb", tag="wv_sb")
        nc.sync.dma_start(wv_sb[:], wv_d.rearrange("(fo fi) g -> fi fo g", fi=P))

        wts = {}
        for nm, wd in (("wq", wq_d), ("wk", wk_d)):
            wt = consts.tile([P, FO, F], bf16, name=f"{nm}t", tag=f"{nm}t")
            nc.sync.dma_start_transpose(wt[:], wd)
            wts[nm] = wt
        m_sb = consts.tile([P, FO, F], bf16, name="m_sb", tag="m_sb")
        for ao in range(FO):
            ps = pp.tile([P, F], f32, name="ps_pp", tag="ps_pp")
            for co in range(FO):
                nc.tensor.matmul(
                    ps[:],
                    wts["wq"][:, co, ao * P : (ao + 1) * P],
                    wts["wk"][:, co, :],
                    start=(co == 0),
                    stop=(co == FO - 1),
                )
            nc.vector.tensor_copy(m_sb[:, ao, :], ps[:])

        slice_list = [sl for _ in range(repeats) for sl in range(n_slices)]
        for s in slice_list:
            # ---- one DRAM x-bar transpose: x[s] [N,F] bf16 -> XT [fi,fo,n] ----
            xt_sb = xt_pool.tile([P, FO, N], bf16, name="xt_sb", tag="xt_sb")
            nc.sync.dma_start_transpose(xt_sb[:], x_d[s])
            dbg = set(debug.split(",")) if debug else set()
            if use_dr:
                xt8_sb = x8_pool.tile([P, FO, N], f8, name="xt8", tag="xt8")
                if "nocasts" not in dbg:
                    for nh in range(MH):
                        nc.gpsimd.tensor_copy(
                            xt8_sb[:, :, nh * 512 : (nh + 1) * 512],
                            xt_sb[:, :, nh * 512 : (nh + 1) * 512],
                        )

            # ---- XMT[g,n] = M.T @ XT (bf16); cast to fp8 for the scores ----
            qt_sb = qt_pool.tile([P, GO, N], bf16, name="qt_sb", tag="qt_sb")
            qt8_sb = q8_pool.tile([P, GO, N], f8, name="qt8", tag="qt8")
            if "goinner" in dbg:
                loop = [(go, nh) for go in range(GO) for nh in range(MH)]
            else:
                # nh outer: each half of qt completes early so its fp8 cast
                # (and the first scores chunks) can start sooner
                loop = [(go, nh) for nh in range(MH) for go in range(GO)]
            for go, nh in loop:
                ps = pp.tile([P, 512], f32, name="ps_pp", tag="ps_pp")
                for fo in range(FO):
                    nc.tensor.matmul(
                        ps[:],
                        m_sb[:, fo, go * P : (go + 1) * P],
                        xt_sb[:, fo, nh * 512 : (nh + 1) * 512],
                        start=(fo == 0),
                        stop=(fo == FO - 1),
                    )
                if "dveqt" in dbg:
                    nc.vector.tensor_copy(
                        qt_sb[:, go, nh * 512 : (nh + 1) * 512], ps[:]
                    )
                else:
                    nc.scalar.activation(
                        qt_sb[:, go, nh * 512 : (nh + 1) * 512], ps[:], COPYF
                    )
                if use_dr and "nocasts" not in dbg and go == GO - 1:
                    nc.vector.tensor_copy(
                        qt8_sb[:, :, nh * 512 : (nh + 1) * 512],
                        qt_sb[:, :, nh * 512 : (nh + 1) * 512],
                    )

            # ---- scores rows / softmax-diag / V / output ----
            for no in range(NO):
                ps = sp.tile([P, MH, 512], f32, name="ps_s", tag="ps_s")
                h, off = divmod(no * P, 512)
                if "noscores" in dbg:
                    pass
                elif use_dr:
                    # bulk scores in fp8 DoubleRow: 2 k-subtiles per matmul
                    for ko in range(0, FO, 2):
                        for hh in range(MH):
                            nc.tensor.matmul(
                                ps[:, hh],
                                qt8_sb[:, ko : ko + 2, no * P : (no + 1) * P],
                                xt8_sb[:, ko : ko + 2, hh * 512 : (hh + 1) * 512],
                                start=(ko == 0),
                                stop=(ko == FO - 2),
                                perf_mode=DR,
                            )
                    # overwrite the 128-wide diagonal stripe in bf16
                    for fo in range(FO):
                        nc.tensor.matmul(
                            ps[:, h, off : off + P],
                            qt_sb[:, fo, no * P : (no + 1) * P],
                            xt_sb[:, fo, no * P : (no + 1) * P],
                            start=(fo == 0),
                            stop=(fo == FO - 1),
                            skip_group_check=True,
                        )
                else:
                    for go in range(GO):
                        lhsT = qt_sb[:, go, no * P : (no + 1) * P]
                        for hh in range(MH):
                            nc.tensor.matmul(
                                ps[:, hh],
                                lhsT,
                                xt_sb[:, go, hh * 512 : (hh + 1) * 512],
                                start=(go == 0),
                                stop=(go == GO - 1),
                            )
                # V row-chunk just-in-time; scaled straight from PSUM
                vtag = "ps_pp" if vp is pp else "ps_v"
                ps_v = vp.tile([P, F], f32, name=vtag, tag=vtag)
                for fo in range(FO):
                    nc.tensor.matmul(
                        ps_v[:],
                        xt_sb[:, fo, no * P : (no + 1) * P],
                        wv_sb[:, fo, :],
                        start=(fo == 0),
                        stop=(fo == FO - 1),
                    )

                ot = outp.tile([P, F], f32, name="ot", tag="ot")
                if "noexp" in dbg or "noscores" in dbg:
                    nc.scalar.activation(ot[:], ps_v[:], COPYF)
                else:
                    # exp over both banks in one pass, fused row-sum
                    ssum = stats.tile([P, 1], f32, name="ssum", tag="ssum")
                    nc.scalar.activation(
                        ps[:], ps[:], EXP, scale=SCALE, accum_out=ssum[:]
                    )
                    # diagonal of exp(scores) via identity mask + reduce
                    dblk = dscr.tile([P, P], f32, name="dblk", tag="dblk")
                    nc.vector.tensor_mul(dblk[:], ps[:, h, off : off + P], ident[:])
                    snn = stats.tile([P, 1], f32, name="snn", tag="snn")
                    nc.vector.tensor_reduce(snn[:], dblk[:], axis=AX, op=OP.add)
                    rec = stats.tile([P, 1], f32, name="rec", tag="rec")
                    nc.vector.reciprocal(rec[:], ssum[:])
                    dval = stats.tile([P, 1], f32, name="dval", tag="dval")
                    nc.vector.tensor_mul(dval[:], snn[:], rec[:])
                    nc.scalar.activation(ot[:], ps_v[:], COPYF, scale=dval[:])
                nc.sync.dma_start(out_d[s, no * P : (no + 1) * P, :], ot[:])

    nc.compile()
    return nc


def build_program_v1(n_slices: int = S, repeats: int = 1, fused_qk: bool = True):
    import concourse.bass as bass
    import concourse.tile as tile
    from concourse import bacc, mybir
    from concourse.masks import make_identity
    from contextlib import ExitStack

    f32 = mybir.dt.float32
    f32r = mybir.dt.float32r
    EXP = mybir.ActivationFunctionType.Exp
    COPYF = mybir.ActivationFunctionType.Identity
    AX = mybir.AxisListType.X
    OP = mybir.AluOpType

    nc = bacc.Bacc(trn_type="TRN2", target_bir_lowering=False, debug=False)
    x_d = nc.dram_tensor("x", [n_slices, N, F], f32, kind="ExternalInput").ap()
    wq_d = nc.dram_tensor("wq", [F, F], f32, kind="ExternalInput").ap()
    wk_d = nc.dram_tensor("wk", [F, F], f32, kind="ExternalInput").ap()
    wv_d = nc.dram_tensor("wv", [F, F], f32, kind="ExternalInput").ap()
    bq_d = nc.dram_tensor("bq", [F], f32, kind="ExternalInput").ap()
    bk_d = nc.dram_tensor("bk", [F], f32, kind="ExternalInput").ap()
    bv_d = nc.dram_tensor("bv", [F], f32, kind="ExternalInput").ap()
    out_d = nc.dram_tensor("out", [n_slices, N, F], f32, kind="ExternalOutput").ap()

    with tile.TileContext(nc) as tc, ExitStack() as ctx:
        consts = ctx.enter_context(tc.tile_pool(name="consts", bufs=1))
        stage = ctx.enter_context(tc.tile_pool(name="stage", bufs=1))
        xin_pool = ctx.enter_context(tc.tile_pool(name="xin", bufs=2))
        xt_pool = ctx.enter_context(tc.tile_pool(name="xt", bufs=2))
        proj_pool = ctx.enter_context(
            tc.tile_pool(name="proj", bufs=2 if fused_qk else 1)
        )
        outp = ctx.enter_context(tc.tile_pool(name="outp", bufs=3))
        dscr = ctx.enter_context(tc.tile_pool(name="dscr", bufs=2))
        stats = ctx.enter_context(tc.tile_pool(name="stats", bufs=6))
        # PSUM budget: 8 banks total = pp(2) + sp(2+2) + tp(2)
        pp = ctx.enter_context(tc.tile_pool(name="pp", bufs=2, space="PSUM"))
        sp = ctx.enter_context(tc.tile_pool(name="sp", bufs=2, space="PSUM"))
        tp = ctx.enter_context(tc.tile_pool(name="tp", bufs=2, space="PSUM"))

        ident = consts.tile([P, P], f32, name="ident", tag="ident")
        make_identity(nc, ident[:])

        def load_and_transpose_x(s, fine_first=False):
            x_sb = xin_pool.tile([P, NO, F], f32, name="x_sb", tag="x_sb")
            x_r = x_d[s].rearrange("(no p) f -> p no f", p=P)
            for no in range(NO):
                if fine_first and no == 0:
                    for fo in range(FO):
                        nc.sync.dma_start(
                            x_sb[:, 0, fo * P : (fo + 1) * P],
                            x_r[:, 0, fo * P : (fo + 1) * P],
                        )
                    continue
                nc.sync.dma_start(x_sb[:, no : no + 1], x_r[:, no : no + 1])
            xt_sb = xt_pool.tile([P, FO, N], f32r, name="xt_sb", tag="xt_sb")
            for no in range(NO):
                t_ps = tp.tile([P, FO, P], f32, name="t_ps", tag="t_ps")
                for fo in range(FO):
                    nc.tensor.transpose(
                        t_ps[:, fo], x_sb[:, no, fo * P : (fo + 1) * P], ident[:]
                    )
                nc.vector.tensor_copy(
                    xt_sb[:, :, no * P : (no + 1) * P], t_ps[:]
                )
            return xt_sb

        slice_list = [sl for _ in range(repeats) for sl in range(n_slices)]
        xt_first = load_and_transpose_x(slice_list[0])

        w_stages = {}
        w_sbs = {}
        for nm, wd in (("wq", wq_d), ("wk", wk_d), ("wv", wv_d)):
            w_stage = stage.tile([P, FO, F], f32, name=f"{nm}_stage", tag=f"{nm}_stage")
            nc.sync.dma_start(w_stage[:], wd.rearrange("(fo fi) g -> fi fo g", fi=P))
            w_stages[nm] = w_stage
            if nm == "wv" or not fused_qk:
                w_sb = consts.tile([P, FO, F], f32r, name=f"{nm}_sb", tag=f"{nm}_sb")
                nc.vector.tensor_copy(w_sb[:], w_stage[:])
                w_sbs[nm] = w_sb
        wv_sb = w_sbs["wv"]

        bq_sb = consts.tile([P, GO], f32, name="bq_sb", tag="bq_sb")
        nc.sync.dma_start(bq_sb[:], bq_d.rearrange("(go gi) -> gi go", gi=P))
        bk_sb = consts.tile([P, GO], f32, name="bk_sb", tag="bk_sb")
        nc.sync.dma_start(bk_sb[:], bk_d.rearrange("(go gi) -> gi go", gi=P))
        bv_bc = consts.tile([P, F], f32, name="bv_bc", tag="bv_bc")
        nc.sync.dma_start(bv_bc[:], bv_d.unsqueeze(0).to_broadcast((P, F)))

        if fused_qk:
            wt_sbs = {}
            for nm in ("wq", "wk"):
                wt_sb = consts.tile([P, FO, F], f32r, name=f"{nm}t_sb", tag=f"{nm}t_sb")
                for ao in range(FO):
                    t_ps = tp.tile([P, FO, P], f32, name="t_ps", tag="t_ps")
                    for co in range(FO):
                        nc.tensor.transpose(
                            t_ps[:, co],
                            w_stages[nm][:, ao, co * P : (co + 1) * P],
                            ident[:],
                        )
                    nc.vector.tensor_copy(
                        wt_sb[:, :, ao * P : (ao + 1) * P], t_ps[:]
                    )
                wt_sbs[nm] = wt_sb
            m_sb = consts.tile([P, FO, F], f32r, name="m_sb", tag="m_sb")
            for ao in range(FO):
                ps = pp.tile([P, F], f32, name="ps_proj", tag="ps_proj")
                for co in range(FO):
                    nc.tensor.matmul(
                        ps[:],
                        wt_sbs["wq"][:, co, ao * P : (ao + 1) * P],
                        wt_sbs["wk"][:, co, :],
                        start=(co == 0),
                        stop=(co == FO - 1),
                    )
                nc.vector.tensor_copy(m_sb[:, ao, :], ps[:])

        for i, s in enumerate(slice_list):
            xt_sb = xt_first if i == 0 else load_and_transpose_x(s)

            if fused_qk:
                qt_sb = proj_pool.tile([P, GO, N], f32r, name="qt_sb", tag="qt_sb")
                kt_sb = xt_sb
                proj_list = ((m_sb, None, qt_sb, True),)
            else:
                v_sb = proj_pool.tile([P, NO, F], f32, name="v_sb", tag="v_sb")
                qt_sb = proj_pool.tile([P, GO, N], f32r, name="qt_sb", tag="qt_sb")
                kt_sb = proj_pool.tile([P, GO, N], f32r, name="kt_sb", tag="kt_sb")
                proj_list = (
                    (w_sbs["wq"], bq_sb, qt_sb, True),
                    (w_sbs["wk"], bk_sb, kt_sb, False),
                )

            for w_sb, b_sb, dst, on_act in proj_list:
                for go in range(GO):
                    for nh in range(MH):
                        ps = pp.tile([P, 512], f32, name="ps_proj", tag="ps_proj")
                        for fo in range(FO):
                            nc.tensor.matmul(
                                ps[:],
                                w_sb[:, fo, go * P : (go + 1) * P],
                                xt_sb[:, fo, nh * 512 : (nh + 1) * 512],
                                start=(fo == 0),
                                stop=(fo == FO - 1),
                            )
                        if on_act:
                            if b_sb is None:
                                nc.scalar.activation(
                                    dst[:, go, nh * 512 : (nh + 1) * 512], ps[:], COPYF
                                )
                            else:
                                nc.scalar.activation(
                                    dst[:, go, nh * 512 : (nh + 1) * 512],
                                    ps[:],
                                    COPYF,
                                    bias=b_sb[:, go : go + 1],
                                )
                        else:
                            nc.vector.tensor_scalar_add(
                                dst[:, go, nh * 512 : (nh + 1) * 512],
                                ps[:],
                                b_sb[:, go : go + 1],
                            )

            if not fused_qk:
                for no in range(NO):
                    ps = pp.tile([P, F], f32, name="ps_proj", tag="ps_proj")
                    for fo in range(FO):
                        nc.tensor.matmul(
                            ps[:],
                            xt_sb[:, fo, no * P : (no + 1) * P],
                            wv_sb[:, fo, :],
                            start=(fo == 0),
                            stop=(fo == FO - 1),
                        )
                    nc.vector.tensor_add(v_sb[:, no, :], ps[:], bv_bc[:])

            for no in range(NO):
                ps0 = sp.tile([P, 512], f32, name="ps_s0", tag="ps_s0")
                ps1 = sp.tile([P, 512], f32, name="ps_s1", tag="ps_s1")
                for go in range(GO):
                    lhsT = qt_sb[:, go, no * P : (no + 1) * P]
                    nc.tensor.matmul(
                        ps0[:], lhsT, kt_sb[:, go, 0:512],
                        start=(go == 0), stop=(go == GO - 1),
                    )
                    nc.tensor.matmul(
                        ps1[:], lhsT, kt_sb[:, go, 512:1024],
                        start=(go == 0), stop=(go == GO - 1),
                    )
                if fused_qk:
                    ps_v = pp.tile([P, F], f32, name="ps_proj", tag="ps_proj")
                    for fo in range(FO):
                        nc.tensor.matmul(
                            ps_v[:],
                            xt_sb[:, fo, no * P : (no + 1) * P],
                            wv_sb[:, fo, :],
                            start=(fo == 0),
                            stop=(fo == FO - 1),
                        )

                s0 = stats.tile([P, 1], f32, name="s0", tag="s0")
                s1 = stats.tile([P, 1], f32, name="s1", tag="s1")
                nc.scalar.activation(
                    ps0[:], ps0[:], EXP, scale=SCALE, accum_out=s0[:]
                )
                nc.scalar.activation(
                    ps1[:], ps1[:], EXP, scale=SCALE, accum_out=s1[:]
                )
                bank, off = divmod(no * P, 512)
                psd = ps0 if bank == 0 else ps1
                dblk = dscr.tile([P, P], f32, name="dblk", tag="dblk")
                nc.vector.tensor_mul(dblk[:], psd[:, off : off + P], ident[:])
                snn = stats.tile([P, 1], f32, name="snn", tag="snn")
                nc.vector.tensor_reduce(snn[:], dblk[:], axis=AX, op=OP.add)

                ssum = stats.tile([P, 1], f32, name="ssum", tag="ssum")
                nc.vector.tensor_add(ssum[:], s0[:], s1[:])
                rec = stats.tile([P, 1], f32, name="rec", tag="rec")
                nc.vector.reciprocal(rec[:], ssum[:])
                dval = stats.tile([P, 1], f32, name="dval", tag="dval")
                nc.vector.tensor_mul(dval[:], snn[:], rec[:])

                ot = outp.tile([P, F], f32, name="ot", tag="ot")
                if fused_qk:
                    nc.scalar.activation(ot[:], ps_v[:], COPYF, scale=dval[:])
                else:
                    nc.vector.tensor_scalar_mul(ot[:], v_sb[:, no, :], dval[:])
                nc.sync.dma_start(out_d[s, no * P : (no + 1) * P, :], ot[:])

    nc.compile()
    return nc


def _get_runner(fused: bool):
    """Build the Bass program once and wrap it in a cached jitted shard_map
    dispatcher (mirrors bass2jax.run_bass_via_pjrt, minus donation so the
    pre-zeroed output operands can be reused across calls — this kernel
    writes every output element)."""
    key = ("runner", fused)
    if key in _CACHE:
        return _CACHE[key]

    import jax
    from jax.experimental.shard_map import shard_map
    from jax.sharding import Mesh, NamedSharding, PartitionSpec
    from concourse import mybir
    from concourse.bass2jax import (
        _bass_exec_p,
        install_neuronx_cc_hook,
        partition_id_tensor,
    )

    nc = build_program(S, fused_qk=fused)
    install_neuronx_cc_hook()
    partition_name = nc.partition_id_tensor.name if nc.partition_id_tensor else None

    in_names, out_names, out_avals, zero_outs = [], [], [], []
    in_dtypes = {}
    for alloc in nc.m.functions[0].allocations:
        if not isinstance(alloc, mybir.MemoryLocationSet):
            continue
        name = alloc.memorylocations[0].name
        if alloc.kind == "ExternalInput":
            if name != partition_name:
                in_names.append(name)
                in_dtypes[name] = mybir.dt.np(alloc.dtype)
        elif alloc.kind == "ExternalOutput":
            shape = tuple(alloc.tensor_shape)
            np_dt = mybir.dt.np(alloc.dtype)
            out_avals.append(jax.core.ShapedArray(shape, np_dt))
            out_names.append(name)
            zero_outs.append(np.zeros(shape, np_dt))

    n_params = len(in_names)
    all_in_names = list(in_names) + list(out_names)
    if partition_name is not None:
        all_in_names.append(partition_name)

    def _body(*args):
        operands = list(args)
        if partition_name is not None:
            operands.append(partition_id_tensor())
        outs = _bass_exec_p.bind(
            *operands,
            out_avals=tuple(out_avals),
            in_names=tuple(all_in_names),
            out_names=tuple(out_names),
            lowering_input_output_aliases=(),
            sim_require_finite=True,
            sim_require_nnan=True,
            nc=nc,
        )
        return tuple(outs)

    devices = jax.devices()[:NCORES]
    mesh = Mesh(np.asarray(devices), ("core",))
    n_outs = len(out_names)
    fn = jax.jit(
        shard_map(
            _body,
            mesh=mesh,
            in_specs=(PartitionSpec("core"),) * (n_params + n_outs),
            out_specs=(PartitionSpec("core"),) * n_outs,
            check_rep=False,
        ),
        keep_unused=True,
    )
    sharding = NamedSharding(mesh, PartitionSpec("core"))
    concat_zeros = [
        jax.device_put(
            np.zeros((NCORES * z.shape[0], *z.shape[1:]), z.dtype), sharding
        )
        for z in zero_outs
    ]
    runner = {
        "fn": fn,
        "in_names": in_names,
        "in_dtypes": in_dtypes,
        "out_names": out_names,
        "zeros": concat_zeros,
        "sharding": sharding,
    }
    _CACHE[key] = runner
    return runner


def kernel(x, Wq, bq, Wk, bk, Wv, bv):
    import jax

    x = np.ascontiguousarray(np.asarray(x, dtype=np.float32))
    shards = x.reshape(B * T, N, F).reshape(NCORES, S, N, F)

    bq = np.ascontiguousarray(np.asarray(bq, dtype=np.float32))
    bk = np.ascontiguousarray(np.asarray(bk, dtype=np.float32))
    bv_arr = np.ascontiguousarray(np.asarray(bv, dtype=np.float32))
    # the fused path assumes zero biases (scores = X (Wq Wk^T) X^T and V
    # scaled straight from PSUM); fall back to the general path otherwise
    fused = bool(not bq.any() and not bk.any() and not bv_arr.any())

    runner = _get_runner(fused)

    per_core = {
        "x": shards.reshape(NCORES * S, N, F),
        "wq": np.tile(np.asarray(Wq, np.float32)[None], (NCORES, 1, 1)).reshape(
            NCORES * F, F
        ),
        "wk": np.tile(np.asarray(Wk, np.float32)[None], (NCORES, 1, 1)).reshape(
            NCORES * F, F
        ),
        "wv": np.tile(np.asarray(Wv, np.float32)[None], (NCORES, 1, 1)).reshape(
            NCORES * F, F
        ),
        "bq": np.tile(bq, NCORES),
        "bk": np.tile(bk, NCORES),
        "bv": np.tile(bv_arr, NCORES),
    }
    def _run(r):
        args = [
            jax.device_put(
                np.ascontiguousarray(
                    np.asarray(per_core[nm]).astype(r["in_dtypes"][nm], copy=False)
                ),
                r["sharding"],
            )
            for nm in r["in_names"]
        ]
        outs = r["fn"](*args, *r["zeros"])
        return np.asarray(outs[r["out_names"].index("out")])

    try:
        out = _run(runner)
    except Exception:
        # stale cached executable/buffers (e.g. device session reset
        # between calls): rebuild once and retry
        _CACHE.pop(("runner", fused), None)
        out = _run(_get_runner(fused))
    return out.reshape(B, T, N, F)


# revision 27
# speedup vs baseline: 1.0627x; 1.0627x over previous
"""GAT-style attention-diagonal kernel for Trainium2 (Bass/Tile), 8-core SPMD.

Reference computation (per (b,t) slice, x:[N,F]):
    Q = x@Wq + bq; K = x@Wk + bk; V = x@Wv + bv
    s = Q @ K.T / sqrt(F)            # [N,N]
    a = softmax(s, axis=-1)
    out = diag(a)[:, None] * V       # only the softmax diagonal is needed

Sharding: data-parallel on the fused B*T axis (48 slices -> 6 per core),
weights replicated.

v2 dataflow (fused path, bq=bk=0), per core per slice:
  - x loaded fp32, cast to bf16 on DVE, transposed to XT [f,n] by the DMA
    x-bar engine (dma_start_transpose; layout matches the [fi,fo] chunk
    convention exactly) -- no PE transposes, no PSUM round trip
  - one-time M = Wq @ Wk.T on device (bf16); XMT[g,n] = M.T @ XT per slice
    (K projection eliminated);  V[n,g] JIT per row chunk
  - scores = XMT.T @ XT: bulk in fp8(e4m3) with DoubleRow perf mode
    (2 k-subtiles per pass -> ~1.8x PE throughput on the N^2 block); the
    128-wide diagonal stripe is then OVERWRITTEN by a bf16 matmul group so
    the softmax numerator exp(s_nn) keeps ~0.3% accuracy while the
    denominator (a 1024-term sum) averages the fp8 noise away
  - exp fused with row-sum on ScalarE over the whole [128,1024] PSUM pair;
    diagonal via identity-mask tensor_tensor_reduce on DVE
  - out row chunk = (exp(s_nn)/rowsum) * V, scaled straight from PSUM

All matmul accumulation is fp32 in PSUM. Expected end-to-end rel err ~0.5%
(tolerance 2e-2).
"""

import numpy as np

B, T, N, F = 4, 12, 1024, 512
NCORES = 8
S = (B * T) // NCORES  # 6 slices per core
P = 128
NO = N // P   # 8 row chunks per slice
FO = F // P   # 4 f chunks
GO = F // P   # 4 g chunks
MH = N // 512  # 2 halves of the scores free axis
SCALE = float(1.0 / np.sqrt(np.float32(F)))

_CACHE: dict = {}


def build_program(n_slices: int = S, repeats: int = 1, fused_qk: bool = True):
    if not fused_qk:
        return build_program_v1(n_slices, repeats, fused_qk)
    return build_program_v2(n_slices, repeats)


def _dedup_ldweights(nc):
    """Drop an InstLdweights when the PE array already holds exactly those
    weights (identical AP + mode, only matmuls in between). The PE sequencer
    is the bottleneck for this kernel (~115ns/instruction), so every removed
    instruction is wall-clock. LDWs carrying semaphore waits/updates are
    kept."""
    from concourse import mybir

    removed = 0
    for b in nc.m.functions[0].blocks:
        insts = b.instructions
        keep = []
        last_sig = None
        for inst in insts:
            nm = type(inst).__name__
            if getattr(inst, "engine", None) != mybir.EngineType.PE:
                keep.append(inst)
                continue
            if nm == "InstLdweights":
                ap = inst.ins[0]
                sig = (
                    str(ap.memref),
                    ap.offset,
                    str(ap.ap),
                    str(ap.dtype),
                    str(inst.perf_mode),
                    str(inst.is_transpose),
                    str(inst.tile_position),
                )
                if sig == last_sig and not inst.has_wait() and not inst.has_update():
                    removed += 1
                    continue
                last_sig = sig
                keep.append(inst)
            elif nm == "InstMatmult":
                keep.append(inst)
            else:
                last_sig = None
                keep.append(inst)
        if len(keep) != len(insts):
            insts[:] = keep
    return removed


def build_program_v2(
    n_slices: int = S,
    repeats: int = 1,
    use_dr: bool = True,
    debug: str = "",
):
    """debug flags (comma-separated, timing-only variants):
    noexp    - skip exp/diag/stats; scale V by 1.0
    noscores - skip the scores matmuls entirely (implies noexp)
    nocasts  - skip the fp8 cast instructions (DR reads uninitialized tiles)
    dveqt    - do the qt PSUM->SBUF bf16 copies on DVE instead of ScalarE
    sp3      - scores PSUM pool bufs=3 and shared V/proj pool (old layout)
    goinner  - projection loop order for go: for nh: (old layout)
    """
    import concourse.bass as bass
    import concourse.tile as tile
    from concourse import bacc, mybir
    from concourse.masks import make_identity
    from contextlib import ExitStack

    f32 = mybir.dt.float32
    bf16 = mybir.dt.bfloat16
    f8 = mybir.dt.float8e4
    EXP = mybir.ActivationFunctionType.Exp
    COPYF = mybir.ActivationFunctionType.Identity
    AX = mybir.AxisListType.X
    OP = mybir.AluOpType
    DR = mybir.MatmulPerfMode.DoubleRow

    nc = bacc.Bacc(trn_type="TRN2", target_bir_lowering=False, debug=False)
    # inputs arrive pre-cast to bf16 (host-side quantize; the kernel's own
    # matmuls are bf16/fp8 anyway) so the x-bar DMA transpose can read DRAM
    # directly (2-byte dtype requirement) and input DMA traffic halves
    x_d = nc.dram_tensor("x", [n_slices, N, F], bf16, kind="ExternalInput").ap()
    wq_d = nc.dram_tensor("wq", [F, F], bf16, kind="ExternalInput").ap()
    wk_d = nc.dram_tensor("wk", [F, F], bf16, kind="ExternalInput").ap()
    wv_d = nc.dram_tensor("wv", [F, F], bf16, kind="ExternalInput").ap()
    out_d = nc.dram_tensor("out", [n_slices, N, F], f32, kind="ExternalOutput").ap()

    with tile.TileContext(nc) as tc, ExitStack() as ctx:
        consts = ctx.enter_context(tc.tile_pool(name="consts", bufs=1))
        xt_pool = ctx.enter_context(tc.tile_pool(name="xt", bufs=2))
        x8_pool = ctx.enter_context(tc.tile_pool(name="x8", bufs=2))
        qt_pool = ctx.enter_context(tc.tile_pool(name="qt", bufs=2))
        q8_pool = ctx.enter_context(tc.tile_pool(name="q8", bufs=2))
        outp = ctx.enter_context(tc.tile_pool(name="outp", bufs=3))
        dscr = ctx.enter_context(tc.tile_pool(name="dscr", bufs=2))
        stats = ctx.enter_context(tc.tile_pool(name="stats", bufs=6))
        dbg0 = set(debug.split(",")) if debug else set()
        if "sp3" in dbg0:
            # 8 banks = pp(2x1, shared proj+V) + sp(3x2)
            pp = ctx.enter_context(tc.tile_pool(name="pp", bufs=2, space="PSUM"))
            vp = pp
            sp = ctx.enter_context(tc.tile_pool(name="sp", bufs=3, space="PSUM"))
        else:
            # 8 banks = pp(2x1 proj) + vp(2x1 V) + sp(2x2 scores)
            pp = ctx.enter_context(tc.tile_pool(name="pp", bufs=2, space="PSUM"))
            vp = ctx.enter_context(tc.tile_pool(name="vp", bufs=2, space="PSUM"))
            sp = ctx.enter_context(tc.tile_pool(name="sp", bufs=2, space="PSUM"))

        ident = consts.tile([P, P], f32, name="ident", tag="ident")
        make_identity(nc, ident[:])

        # ---- weights: Wv loaded [fi,fo,g]; WqT/WkT via one DRAM x-bar
        # transpose each; then M = Wq @ Wk.T on PE ----
        wv_sb = consts.tile([P, FO, F], bf16, name="wv_sb", tag="wv_sb")
        nc.sync.dma_start(wv_sb[:], wv_d.rearrange("(fo fi) g -> fi fo g", fi=P))

        wts = {}
        for nm, wd in (("wq", wq_d), ("wk", wk_d)):
            wt = consts.tile([P, FO, F], bf16, name=f"{nm}t", tag=f"{nm}t")
            nc.sync.dma_start_transpose(wt[:], wd)
            wts[nm] = wt
        m_sb = consts.tile([P, FO, F], bf16, name="m_sb", tag="m_sb")
        for ao in range(FO):
            ps = pp.tile([P, F], f32, name="ps_pp", tag="ps_pp")
            for co in range(FO):
                nc.tensor.matmul(
                    ps[:],
                    wts["wq"][:, co, ao * P : (ao + 1) * P],
                    wts["wk"][:, co, :],
                    start=(co == 0),
                    stop=(co == FO - 1),
                )
            nc.vector.tensor_copy(m_sb[:, ao, :], ps[:])

        slice_list = [sl for _ in range(repeats) for sl in range(n_slices)]
        for s in slice_list:
            # ---- one DRAM x-bar transpose: x[s] [N,F] bf16 -> XT [fi,fo,n] ----
            xt_sb = xt_pool.tile([P, FO, N], bf16, name="xt_sb", tag="xt_sb")
            nc.sync.dma_start_transpose(xt_sb[:], x_d[s])
            dbg = set(debug.split(",")) if debug else set()
            if use_dr:
                xt8_sb = x8_pool.tile([P, FO, N], f8, name="xt8", tag="xt8")
                if "nocasts" not in dbg:
                    for nh in range(MH):
                        nc.gpsimd.tensor_copy(
                            xt8_sb[:, :, nh * 512 : (nh + 1) * 512],
                            xt_sb[:, :, nh * 512 : (nh + 1) * 512],
                        )

            # ---- XMT[g,n] = M.T @ XT (bf16); cast to fp8 for the scores ----
            # both n-halves accumulate in parallel PSUM tiles so each M-chunk
            # lhsT is loaded once for two matmuls (LDWEIGHTS dedup target)
            qt_sb = qt_pool.tile([P, GO, N], bf16, name="qt_sb", tag="qt_sb")
            qt8_sb = q8_pool.tile([P, GO, N], f8, name="qt8", tag="qt8")
            for go in range(GO):
                ps0 = pp.tile([P, 512], f32, name="ps_pp", tag="ps_pp")
                vtag = "ps_pp" if vp is pp else "ps_v"
                ps1 = vp.tile([P, 512], f32, name=vtag, tag=vtag)
                for fo in range(FO):
                    m_lhsT = m_sb[:, fo, go * P : (go + 1) * P]
                    nc.tensor.matmul(
                        ps0[:], m_lhsT, xt_sb[:, fo, 0:512],
                        start=(fo == 0), stop=(fo == FO - 1),
                    )
                    nc.tensor.matmul(
                        ps1[:], m_lhsT, xt_sb[:, fo, 512:1024],
                        start=(fo == 0), stop=(fo == FO - 1),
                    )
                for nh, psn in ((0, ps0), (1, ps1)):
                    if "actqt" in dbg:
                        nc.scalar.activation(
                            qt_sb[:, go, nh * 512 : (nh + 1) * 512], psn[:], COPYF
                        )
                    else:
                        nc.vector.tensor_copy(
                            qt_sb[:, go, nh * 512 : (nh + 1) * 512], psn[:]
                        )
            if use_dr and "nocasts" not in dbg:
                for nh in range(MH):
                    nc.vector.tensor_copy(
                        qt8_sb[:, :, nh * 512 : (nh + 1) * 512],
                        qt_sb[:, :, nh * 512 : (nh + 1) * 512],
                    )

            # ---- scores rows / softmax-diag / V / output ----
            for no in range(NO):
                ps = sp.tile([P, MH, 512], f32, name="ps_s", tag="ps_s")
                h, off = divmod(no * P, 512)
                if "noscores" in dbg:
                    pass
                elif use_dr:
                    # bulk scores in fp8 DoubleRow: 2 k-subtiles per matmul,
                    # each lhsT shared by both halves (LDWEIGHTS dedup)
                    for ko in range(0, FO, 2):
                        for hh in range(MH):
                            nc.tensor.matmul(
                                ps[:, hh],
                                qt8_sb[:, ko : ko + 2, no * P : (no + 1) * P],
                                xt8_sb[:, ko : ko + 2, hh * 512 : (hh + 1) * 512],
                                start=(ko == 0),
                                stop=(ko == FO - 2),
                                perf_mode=DR,
                            )
                else:
                    for go in range(GO):
                        lhsT = qt_sb[:, go, no * P : (no + 1) * P]
                        for hh in range(MH):
                            nc.tensor.matmul(
                                ps[:, hh],
                                lhsT,
                                xt_sb[:, go, hh * 512 : (hh + 1) * 512],
                                start=(go == 0),
                                stop=(go == GO - 1),
                            )

                # V row chunk and (in DR mode) the transposed bf16 diagonal
                # block S_blk.T = XT.T @ XMT share each xt-chunk lhsT; the
                # diagonal is transpose-invariant, so diag(S_blk.T)=diag(S_blk)
                vtag = "ps_pp" if vp is pp else "ps_v"
                ps_v = vp.tile([P, F], f32, name=vtag, tag=vtag)
                st_ps = pp.tile([P, 512], f32, name="ps_pp", tag="ps_pp")
                for fo in range(FO):
                    v_lhsT = xt_sb[:, fo, no * P : (no + 1) * P]
                    if use_dr and "noscores" not in dbg:
                        nc.tensor.matmul(
                            st_ps[:, 0:P],
                            v_lhsT,
                            qt_sb[:, fo, no * P : (no + 1) * P],
                            start=(fo == 0),
                            stop=(fo == FO - 1),
                        )
                    nc.tensor.matmul(
                        ps_v[:],
                        v_lhsT,
                        wv_sb[:, fo, :],
                        start=(fo == 0),
                        stop=(fo == FO - 1),
                    )

                ot = outp.tile([P, F], f32, name="ot", tag="ot")
                if "noexp" in dbg or "noscores" in dbg:
                    nc.scalar.activation(ot[:], ps_v[:], COPYF)
                else:
                    # exp over both banks in one pass, fused row-sum
                    ssum = stats.tile([P, 1], f32, name="ssum", tag="ssum")
                    nc.scalar.activation(
                        ps[:], ps[:], EXP, scale=SCALE, accum_out=ssum[:]
                    )
                    dblk = dscr.tile([P, P], f32, name="dblk", tag="dblk")
                    snn = stats.tile([P, 1], f32, name="snn", tag="snn")
                    if use_dr:
                        # exact diagonal from the bf16 block: mask + reduce
                        # the raw scores, then exp on the [P,1] result
                        nc.vector.tensor_mul(dblk[:], st_ps[:, 0:P], ident[:])
                        snn_r = stats.tile([P, 1], f32, name="snn_r", tag="snn_r")
                        nc.vector.tensor_reduce(snn_r[:], dblk[:], axis=AX, op=OP.add)
                        nc.scalar.activation(snn[:], snn_r[:], EXP, scale=SCALE)
                    else:
                        nc.vector.tensor_mul(
                            dblk[:], ps[:, h, off : off + P], ident[:]
                        )
                        nc.vector.tensor_reduce(snn[:], dblk[:], axis=AX, op=OP.add)
                    rec = stats.tile([P, 1], f32, name="rec", tag="rec")
                    nc.vector.reciprocal(rec[:], ssum[:])
                    dval = stats.tile([P, 1], f32, name="dval", tag="dval")
                    nc.vector.tensor_mul(dval[:], snn[:], rec[:])
                    nc.scalar.activation(ot[:], ps_v[:], COPYF, scale=dval[:])
                nc.sync.dma_start(out_d[s, no * P : (no + 1) * P, :], ot[:])

    nc.compile()
    if "nodedup" not in (set(debug.split(",")) if debug else set()):
        _dedup_ldweights(nc)
    return nc


def build_program_v1(n_slices: int = S, repeats: int = 1, fused_qk: bool = True):
    import concourse.bass as bass
    import concourse.tile as tile
    from concourse import bacc, mybir
    from concourse.masks import make_identity
    from contextlib import ExitStack

    f32 = mybir.dt.float32
    f32r = mybir.dt.float32r
    EXP = mybir.ActivationFunctionType.Exp
    COPYF = mybir.ActivationFunctionType.Identity
    AX = mybir.AxisListType.X
    OP = mybir.AluOpType

    nc = bacc.Bacc(trn_type="TRN2", target_bir_lowering=False, debug=False)
    x_d = nc.dram_tensor("x", [n_slices, N, F], f32, kind="ExternalInput").ap()
    wq_d = nc.dram_tensor("wq", [F, F], f32, kind="ExternalInput").ap()
    wk_d = nc.dram_tensor("wk", [F, F], f32, kind="ExternalInput").ap()
    wv_d = nc.dram_tensor("wv", [F, F], f32, kind="ExternalInput").ap()
    bq_d = nc.dram_tensor("bq", [F], f32, kind="ExternalInput").ap()
    bk_d = nc.dram_tensor("bk", [F], f32, kind="ExternalInput").ap()
    bv_d = nc.dram_tensor("bv", [F], f32, kind="ExternalInput").ap()
    out_d = nc.dram_tensor("out", [n_slices, N, F], f32, kind="ExternalOutput").ap()

    with tile.TileContext(nc) as tc, ExitStack() as ctx:
        consts = ctx.enter_context(tc.tile_pool(name="consts", bufs=1))
        stage = ctx.enter_context(tc.tile_pool(name="stage", bufs=1))
        xin_pool = ctx.enter_context(tc.tile_pool(name="xin", bufs=2))
        xt_pool = ctx.enter_context(tc.tile_pool(name="xt", bufs=2))
        proj_pool = ctx.enter_context(
            tc.tile_pool(name="proj", bufs=2 if fused_qk else 1)
        )
        outp = ctx.enter_context(tc.tile_pool(name="outp", bufs=3))
        dscr = ctx.enter_context(tc.tile_pool(name="dscr", bufs=2))
        stats = ctx.enter_context(tc.tile_pool(name="stats", bufs=6))
        # PSUM budget: 8 banks total = pp(2) + sp(2+2) + tp(2)
        pp = ctx.enter_context(tc.tile_pool(name="pp", bufs=2, space="PSUM"))
        sp = ctx.enter_context(tc.tile_pool(name="sp", bufs=2, space="PSUM"))
        tp = ctx.enter_context(tc.tile_pool(name="tp", bufs=2, space="PSUM"))

        ident = consts.tile([P, P], f32, name="ident", tag="ident")
        make_identity(nc, ident[:])

        def load_and_transpose_x(s, fine_first=False):
            x_sb = xin_pool.tile([P, NO, F], f32, name="x_sb", tag="x_sb")
            x_r = x_d[s].rearrange("(no p) f -> p no f", p=P)
            for no in range(NO):
                if fine_first and no == 0:
                    for fo in range(FO):
                        nc.sync.dma_start(
                            x_sb[:, 0, fo * P : (fo + 1) * P],
                            x_r[:, 0, fo * P : (fo + 1) * P],
                        )
                    continue
                nc.sync.dma_start(x_sb[:, no : no + 1], x_r[:, no : no + 1])
            xt_sb = xt_pool.tile([P, FO, N], f32r, name="xt_sb", tag="xt_sb")
            for no in range(NO):
                t_ps = tp.tile([P, FO, P], f32, name="t_ps", tag="t_ps")
                for fo in range(FO):
                    nc.tensor.transpose(
                        t_ps[:, fo], x_sb[:, no, fo * P : (fo + 1) * P], ident[:]
                    )
                nc.vector.tensor_copy(
                    xt_sb[:, :, no * P : (no + 1) * P], t_ps[:]
                )
            return xt_sb

        slice_list = [sl for _ in range(repeats) for sl in range(n_slices)]
        xt_first = load_and_transpose_x(slice_list[0])

        w_stages = {}
        w_sbs = {}
        for nm, wd in (("wq", wq_d), ("wk", wk_d), ("wv", wv_d)):
            w_stage = stage.tile([P, FO, F], f32, name=f"{nm}_stage", tag=f"{nm}_stage")
            nc.sync.dma_start(w_stage[:], wd.rearrange("(fo fi) g -> fi fo g", fi=P))
            w_stages[nm] = w_stage
            if nm == "wv" or not fused_qk:
                w_sb = consts.tile([P, FO, F], f32r, name=f"{nm}_sb", tag=f"{nm}_sb")
                nc.vector.tensor_copy(w_sb[:], w_stage[:])
                w_sbs[nm] = w_sb
        wv_sb = w_sbs["wv"]

        bq_sb = consts.tile([P, GO], f32, name="bq_sb", tag="bq_sb")
        nc.sync.dma_start(bq_sb[:], bq_d.rearrange("(go gi) -> gi go", gi=P))
        bk_sb = consts.tile([P, GO], f32, name="bk_sb", tag="bk_sb")
        nc.sync.dma_start(bk_sb[:], bk_d.rearrange("(go gi) -> gi go", gi=P))
        bv_bc = consts.tile([P, F], f32, name="bv_bc", tag="bv_bc")
        nc.sync.dma_start(bv_bc[:], bv_d.unsqueeze(0).to_broadcast((P, F)))

        if fused_qk:
            wt_sbs = {}
            for nm in ("wq", "wk"):
                wt_sb = consts.tile([P, FO, F], f32r, name=f"{nm}t_sb", tag=f"{nm}t_sb")
                for ao in range(FO):
                    t_ps = tp.tile([P, FO, P], f32, name="t_ps", tag="t_ps")
                    for co in range(FO):
                        nc.tensor.transpose(
                            t_ps[:, co],
                            w_stages[nm][:, ao, co * P : (co + 1) * P],
                            ident[:],
                        )
                    nc.vector.tensor_copy(
                        wt_sb[:, :, ao * P : (ao + 1) * P], t_ps[:]
                    )
                wt_sbs[nm] = wt_sb
            m_sb = consts.tile([P, FO, F], f32r, name="m_sb", tag="m_sb")
            for ao in range(FO):
                ps = pp.tile([P, F], f32, name="ps_proj", tag="ps_proj")
                for co in range(FO):
                    nc.tensor.matmul(
                        ps[:],
                        wt_sbs["wq"][:, co, ao * P : (ao + 1) * P],
                        wt_sbs["wk"][:, co, :],
                        start=(co == 0),
                        stop=(co == FO - 1),
                    )
                nc.vector.tensor_copy(m_sb[:, ao, :], ps[:])

        for i, s in enumerate(slice_list):
            xt_sb = xt_first if i == 0 else load_and_transpose_x(s)

            if fused_qk:
                qt_sb = proj_pool.tile([P, GO, N], f32r, name="qt_sb", tag="qt_sb")
                kt_sb = xt_sb
                proj_list = ((m_sb, None, qt_sb, True),)
            else:
                v_sb = proj_pool.tile([P, NO, F], f32, name="v_sb", tag="v_sb")
                qt_sb = proj_pool.tile([P, GO, N], f32r, name="qt_sb", tag="qt_sb")
                kt_sb = proj_pool.tile([P, GO, N], f32r, name="kt_sb", tag="kt_sb")
                proj_list = (
                    (w_sbs["wq"], bq_sb, qt_sb, True),
                    (w_sbs["wk"], bk_sb, kt_sb, False),
                )

            for w_sb, b_sb, dst, on_act in proj_list:
                for go in range(GO):
                    for nh in range(MH):
                        ps = pp.tile([P, 512], f32, name="ps_proj", tag="ps_proj")
                        for fo in range(FO):
                            nc.tensor.matmul(
                                ps[:],
                                w_sb[:, fo, go * P : (go + 1) * P],
                                xt_sb[:, fo, nh * 512 : (nh + 1) * 512],
                                start=(fo == 0),
                                stop=(fo == FO - 1),
                            )
                        if on_act:
                            if b_sb is None:
                                nc.scalar.activation(
                                    dst[:, go, nh * 512 : (nh + 1) * 512], ps[:], COPYF
                                )
                            else:
                                nc.scalar.activation(
                                    dst[:, go, nh * 512 : (nh + 1) * 512],
                                    ps[:],
                                    COPYF,
                                    bias=b_sb[:, go : go + 1],
                                )
                        else:
                            nc.vector.tensor_scalar_add(
                                dst[:, go, nh * 512 : (nh + 1) * 512],
                                ps[:],
                                b_sb[:, go : go + 1],
                            )

            if not fused_qk:
                for no in range(NO):
                    ps = pp.tile([P, F], f32, name="ps_proj", tag="ps_proj")
                    for fo in range(FO):
                        nc.tensor.matmul(
                            ps[:],
                            xt_sb[:, fo, no * P : (no + 1) * P],
                            wv_sb[:, fo, :],
                            start=(fo == 0),
                            stop=(fo == FO - 1),
                        )
                    nc.vector.tensor_add(v_sb[:, no, :], ps[:], bv_bc[:])

            for no in range(NO):
                ps0 = sp.tile([P, 512], f32, name="ps_s0", tag="ps_s0")
                ps1 = sp.tile([P, 512], f32, name="ps_s1", tag="ps_s1")
                for go in range(GO):
                    lhsT = qt_sb[:, go, no * P : (no + 1) * P]
                    nc.tensor.matmul(
                        ps0[:], lhsT, kt_sb[:, go, 0:512],
                        start=(go == 0), stop=(go == GO - 1),
                    )
                    nc.tensor.matmul(
                        ps1[:], lhsT, kt_sb[:, go, 512:1024],
                        start=(go == 0), stop=(go == GO - 1),
                    )
                if fused_qk:
                    ps_v = pp.tile([P, F], f32, name="ps_proj", tag="ps_proj")
                    for fo in range(FO):
                        nc.tensor.matmul(
                            ps_v[:],
                            xt_sb[:, fo, no * P : (no + 1) * P],
                            wv_sb[:, fo, :],
                            start=(fo == 0),
                            stop=(fo == FO - 1),
                        )

                s0 = stats.tile([P, 1], f32, name="s0", tag="s0")
                s1 = stats.tile([P, 1], f32, name="s1", tag="s1")
                nc.scalar.activation(
                    ps0[:], ps0[:], EXP, scale=SCALE, accum_out=s0[:]
                )
                nc.scalar.activation(
                    ps1[:], ps1[:], EXP, scale=SCALE, accum_out=s1[:]
                )
                bank, off = divmod(no * P, 512)
                psd = ps0 if bank == 0 else ps1
                dblk = dscr.tile([P, P], f32, name="dblk", tag="dblk")
                nc.vector.tensor_mul(dblk[:], psd[:, off : off + P], ident[:])
                snn = stats.tile([P, 1], f32, name="snn", tag="snn")
                nc.vector.tensor_reduce(snn[:], dblk[:], axis=AX, op=OP.add)

                ssum = stats.tile([P, 1], f32, name="ssum", tag="ssum")
                nc.vector.tensor_add(ssum[:], s0[:], s1[:])
                rec = stats.tile([P, 1], f32, name="rec", tag="rec")
                nc.vector.reciprocal(rec[:], ssum[:])
                dval = stats.tile([P, 1], f32, name="dval", tag="dval")
                nc.vector.tensor_mul(dval[:], snn[:], rec[:])

                ot = outp.tile([P, F], f32, name="ot", tag="ot")
                if fused_qk:
                    nc.scalar.activation(ot[:], ps_v[:], COPYF, scale=dval[:])
                else:
                    nc.vector.tensor_scalar_mul(ot[:], v_sb[:, no, :], dval[:])
                nc.sync.dma_start(out_d[s, no * P : (no + 1) * P, :], ot[:])

    nc.compile()
    return nc


def _get_runner(fused: bool):
    """Build the Bass program once and wrap it in a cached jitted shard_map
    dispatcher (mirrors bass2jax.run_bass_via_pjrt, minus donation so the
    pre-zeroed output operands can be reused across calls — this kernel
    writes every output element)."""
    key = ("runner", fused)
    if key in _CACHE:
        return _CACHE[key]

    import jax
    from jax.experimental.shard_map import shard_map
    from jax.sharding import Mesh, NamedSharding, PartitionSpec
    from concourse import mybir
    from concourse.bass2jax import (
        _bass_exec_p,
        install_neuronx_cc_hook,
        partition_id_tensor,
    )

    nc = build_program(S, fused_qk=fused)
    install_neuronx_cc_hook()
    partition_name = nc.partition_id_tensor.name if nc.partition_id_tensor else None

    in_names, out_names, out_avals, zero_outs = [], [], [], []
    in_dtypes = {}
    for alloc in nc.m.functions[0].allocations:
        if not isinstance(alloc, mybir.MemoryLocationSet):
            continue
        name = alloc.memorylocations[0].name
        if alloc.kind == "ExternalInput":
            if name != partition_name:
                in_names.append(name)
                in_dtypes[name] = mybir.dt.np(alloc.dtype)
        elif alloc.kind == "ExternalOutput":
            shape = tuple(alloc.tensor_shape)
            np_dt = mybir.dt.np(alloc.dtype)
            out_avals.append(jax.core.ShapedArray(shape, np_dt))
            out_names.append(name)
            zero_outs.append(np.zeros(shape, np_dt))

    n_params = len(in_names)
    all_in_names = list(in_names) + list(out_names)
    if partition_name is not None:
        all_in_names.append(partition_name)

    def _body(*args):
        operands = list(args)
        if partition_name is not None:
            operands.append(partition_id_tensor())
        outs = _bass_exec_p.bind(
            *operands,
            out_avals=tuple(out_avals),
            in_names=tuple(all_in_names),
            out_names=tuple(out_names),
            lowering_input_output_aliases=(),
            sim_require_finite=True,
            sim_require_nnan=True,
            nc=nc,
        )
        return tuple(outs)

    devices = jax.devices()[:NCORES]
    mesh = Mesh(np.asarray(devices), ("core",))
    n_outs = len(out_names)
    fn = jax.jit(
        shard_map(
            _body,
            mesh=mesh,
            in_specs=(PartitionSpec("core"),) * (n_params + n_outs),
            out_specs=(PartitionSpec("core"),) * n_outs,
            check_rep=False,
        ),
        keep_unused=True,
    )
    sharding = NamedSharding(mesh, PartitionSpec("core"))
    concat_zeros = [
        jax.device_put(
            np.zeros((NCORES * z.shape[0], *z.shape[1:]), z.dtype), sharding
        )
        for z in zero_outs
    ]
    runner = {
        "fn": fn,
        "in_names": in_names,
        "in_dtypes": in_dtypes,
        "out_names": out_names,
        "zeros": concat_zeros,
        "sharding": sharding,
    }
    _CACHE[key] = runner
    return runner


def kernel(x, Wq, bq, Wk, bk, Wv, bv):
    import jax

    x = np.ascontiguousarray(np.asarray(x, dtype=np.float32))
    shards = x.reshape(B * T, N, F).reshape(NCORES, S, N, F)

    bq = np.ascontiguousarray(np.asarray(bq, dtype=np.float32))
    bk = np.ascontiguousarray(np.asarray(bk, dtype=np.float32))
    bv_arr = np.ascontiguousarray(np.asarray(bv, dtype=np.float32))
    # the fused path assumes zero biases (scores = X (Wq Wk^T) X^T and V
    # scaled straight from PSUM); fall back to the general path otherwise
    fused = bool(not bq.any() and not bk.any() and not bv_arr.any())

    runner = _get_runner(fused)

    per_core = {
        "x": shards.reshape(NCORES * S, N, F),
        "wq": np.tile(np.asarray(Wq, np.float32)[None], (NCORES, 1, 1)).reshape(
            NCORES * F, F
        ),
        "wk": np.tile(np.asarray(Wk, np.float32)[None], (NCORES, 1, 1)).reshape(
            NCORES * F, F
        ),
        "wv": np.tile(np.asarray(Wv, np.float32)[None], (NCORES, 1, 1)).reshape(
            NCORES * F, F
        ),
        "bq": np.tile(bq, NCORES),
        "bk": np.tile(bk, NCORES),
        "bv": np.tile(bv_arr, NCORES),
    }
    def _run(r):
        args = [
            jax.device_put(
                np.ascontiguousarray(
                    np.asarray(per_core[nm]).astype(r["in_dtypes"][nm], copy=False)
                ),
                r["sharding"],
            )
            for nm in r["in_names"]
        ]
        outs = r["fn"](*args, *r["zeros"])
        return np.asarray(outs[r["out_names"].index("out")])

    try:
        out = _run(runner)
    except Exception:
        # stale cached executable/buffers (e.g. device session reset
        # between calls): rebuild once and retry
        _CACHE.pop(("runner", fused), None)
        out = _run(_get_runner(fused))
    return out.reshape(B, T, N, F)
